# revision 18
# baseline (speedup 1.0000x reference)
"""MoE layer (B=8,T=1024,D=512,F=2048,E=8,top-2) on 8 NeuronCores.

Strategy (expert parallel, per the sharding hint):
- Host computes the router (logits -> softmax -> top-2 -> combine weights);
  tokens are gathered per expert and dispatched to the core owning that
  expert (the "all-to-all by routing assignment" is the host gather).
- Core e runs the expert-e FFN over its gathered tokens, split in two
  precision classes by combine weight: the C8 tokens with the SMALLEST
  combine weights run both matmuls in fp8-e4m3 DoubleRow mode (~1.45x PE
  rate; their output error is scaled by the small cw, keeping total
  rel-err under the 2e-2 gate), the remaining C16 tokens run in fp16.
- All device tensors are shipped pre-swizzled into SBUF layout ([128
  partitions, free]) so every DMA is one long contiguous run per
  partition: the DMA rings here are packet-rate limited, and the naive
  [(k p) f] strided loads cost 4x the packets for the same bytes.
- Host scatter-adds the per-expert outputs back (plus the cw-weighted b2
  rank-1 term); the ~1.5% capacity-overflow tail of highest-cw tokens is
  computed exactly in fp32 on the host, as in the baseline.
"""

import os
import numpy as np
import ml_dtypes

import concourse.bass as bass
from bass_rust import add_dep_helper
import concourse.tile as tile
from concourse import bacc, mybir
from concourse.bass_utils import run_bass_kernel_spmd

F32 = mybir.dt.float32
F16 = mybir.dt.float16
F8 = mybir.dt.float8e4
NP_F8 = ml_dtypes.float8_e4m3  # TRN e4m3 (max normal 240)

B, T, D, F, E, TOPK = 8, 1024, 512, 2048, 8, 2
N = B * T
P = 128
N_CORES = 8
KT1 = D // P    # 4  k-tiles for x @ W1
KT2 = F // P    # 16 k-tiles for h @ W2
FT = F // P     # 16 f-tiles of hT

C = 2048        # device token capacity per expert (= N*TOPK / E exactly)
C8 = 384        # lowest-cw tokens per expert in fp8 DoubleRow
C16 = C - C8    # fp16 tokens per expert
CT = C // P

# fp8 scales (powers of two, folded exactly):
#   x8 = q8(x*16), w1_8 = q8(W1*4096)  -> psum = (x@W1) * 2^16
#   h8 = relu(psum * 2^-16 + b1)       -> h in natural units, fp8
#   w2_8 = q8(W2*8192)                 -> psum = y * 2^13
#   y = psum * (cw * 2^-13)            -> cw pre-scaled on host
SX, SW1, SW2 = 16.0, 4096.0, 8192.0
H_SCALE = 1.0 / 65536.0
CW8_SCALE = 1.0 / 8192.0

CHUNKS16 = [(0, 384), (384, 512), (896, 512), (1408, 256)]  # sums to C16
XT_OFF = []
_o = 0
for _c0, _s in CHUNKS16:
    XT_OFF.append(_o)
    _o += KT1 * _s
XT_COLS = _o            # 4*C16
N_WARM = 16     # PE clock-ramp matmuls before real data lands
N_TAILJUNK = 20  # post-work matmuls keeping HAM at 8/8 through the epilogue

_BUILD_CACHE = {}


def _build():
    if "nc" in _BUILD_CACHE:
        return _BUILD_CACHE["nc"]
    nc = bacc.Bacc()

    # All inputs are host-preswizzled to SBUF layout [128, free].
    xt_d = nc.dram_tensor("xt", [P, XT_COLS], F16, kind="ExternalInput")
    x8_d = nc.dram_tensor("x8", [P, 4 * C8], F8, kind="ExternalInput")
    w1_d = nc.dram_tensor("w1", [P, KT1 * F], F16, kind="ExternalInput")
    w2_d = nc.dram_tensor("w2", [P, KT2 * D], F16, kind="ExternalInput")
    w18_d = nc.dram_tensor("w18", [P, KT1 * F], F8, kind="ExternalInput")
    w28_d = nc.dram_tensor("w28", [P, KT2 * D], F8, kind="ExternalInput")
    # b1 in cols 0:FT, cw in cols FT:FT+CT (cw cols for fp8 tokens pre-scaled)
    bc_d = nc.dram_tensor("bc", [P, FT + CT], F32, kind="ExternalInput")
    y_d = nc.dram_tensor("y", [C, D], F16, kind="ExternalOutput")

    with tile.TileContext(nc) as tc:
        with (
            tc.tile_pool(name="weights", bufs=1) as wpool,
            tc.tile_pool(name="xt", bufs=1) as xpool,
            tc.tile_pool(name="h", bufs=2 * FT + 1) as hpool,
            tc.tile_pool(name="y", bufs=6) as ypool,
            tc.tile_pool(name="psh", bufs=3, space="PSUM") as psh,
            tc.tile_pool(name="psy", bufs=3, space="PSUM") as psy,
        ):
            # ---- tiles (layouts match the host swizzle exactly) ----
            # w1:  [p, h(2), kt(4), fw(1024)]  h = fi//8, fw = (fi%8)*128 + ...
            w1_t = wpool.tile([P, KT1 * F], F16, tag="w1")
            w1_v = w1_t[:].rearrange("p (h kt fw) -> p h kt fw", h=2, kt=KT1)
            # w18: [p, h(2), q(2), j(2), fw(1024)]
            w18_t = wpool.tile([P, KT1 * F], F8, tag="w18")
            w18_v = w18_t[:].rearrange(
                "p (h q j fw) -> p h q j fw", h=2, q=2, j=2
            )
            # w2:  [p, kt(16), d(512)]
            w2_t = wpool.tile([P, KT2 * D], F16, tag="w2")
            # w28: [p, g(8), j(2), d(512)]
            w28_t = wpool.tile([P, KT2 * D], F8, tag="w28")
            w28_v = w28_t[:].rearrange("p (g j d) -> p g j d", g=8, j=2)
            bc_t = wpool.tile([P, FT + CT], F32, tag="bc")
            # xt: per chunk [p, kt(4), c(S)] packed back to back
            xt_t = xpool.tile([P, XT_COLS], F16, tag="xt")
            xt_chunk = [
                xt_t[:, XT_OFF[ci] : XT_OFF[ci] + KT1 * S].rearrange(
                    "p (kt c) -> p kt c", kt=KT1
                )
                for ci, (_, S) in enumerate(CHUNKS16)
            ]
            # x8: [p, q(2), j(2), c(C8)]
            x8_t = xpool.tile([P, 4 * C8], F8, tag="x8")
            x8_v = x8_t[:].rearrange("p (q j c) -> p q j c", q=2, j=2)
            h8_t = wpool.tile([P, FT * C8], F8, tag="h8")
            h8_v = h8_t[:].rearrange("p (g j c) -> p g j c", g=8, j=2)

            # PE warm-up: junk matmuls on a zeroed tile while the input DMAs
            # stream, so the HAM clock-gate reaches 8/8 before real work.
            warm = wpool.tile([P, 512], F16, tag="warm")
            nc.vector.memset(warm[:], 0.0)
            wps = psy.tile([P, 512], F32, tag="psy")
            for _ in range(N_WARM):
                nc.tensor.matmul(wps[:], warm[:, 0:P], warm[:], start=True, stop=True)

            # ---- DMA plan ----
            # fp8 phase runs first: its critical inputs (x8 + w18 low half)
            # land while the warm-up ramps the clock. Every DMA below is one
            # contiguous <=4KB run per partition.
            nc.sync.dma_start(w18_t[:, KT1 * F // 2 :], w18_d[:, KT1 * F // 2 :])
            s0, e0 = XT_OFF[0], XT_OFF[0] + KT1 * CHUNKS16[0][1]
            nc.sync.dma_start(xt_t[:, s0:e0], xt_d[:, s0:e0])
            # w1 high half (fi8-15) in two 4KB pieces
            nc.sync.dma_start(w1_t[:, 4096:6144], w1_d[:, 4096:6144])
            nc.sync.dma_start(w1_t[:, 6144:8192], w1_d[:, 6144:8192])
            for ci in range(1, len(CHUNKS16)):
                s, e = XT_OFF[ci], XT_OFF[ci] + KT1 * CHUNKS16[ci][1]
                nc.sync.dma_start(xt_t[:, s:e], xt_d[:, s:e])

            # x8 + w18 low half on this queue so both halves of the fp8
            # critical path stream concurrently with w18's high half
            nc.gpsimd.dma_start(x8_t[:], x8_d[:])
            nc.gpsimd.dma_start(
                w18_t[:, 0 : KT1 * F // 2], w18_d[:, 0 : KT1 * F // 2]
            )
            nc.gpsimd.dma_start(bc_t[:], bc_d[:])
            # w1 low half (fi0-7) in two 4KB pieces
            nc.gpsimd.dma_start(w1_t[:, 0:2048], w1_d[:, 0:2048])
            nc.gpsimd.dma_start(w1_t[:, 2048:4096], w1_d[:, 2048:4096])
            # w28 (8KB) in two pieces, then w2 (16KB) in four pieces
            nc.gpsimd.dma_start(w28_t[:, 0:4096], w28_d[:, 0:4096])
            nc.gpsimd.dma_start(w28_t[:, 4096:8192], w28_d[:, 4096:8192])
            for k in range(4):
                nc.gpsimd.dma_start(
                    w2_t[:, k * 2048 : (k + 1) * 2048],
                    w2_d[:, k * 2048 : (k + 1) * 2048],
                )

            # ---- PE group-order chain (pin issue order to program order) ----
            h_tiles = {}
            prev_grp = [None, None]

            def group_start():
                prev_grp[0], prev_grp[1] = prev_grp[1], None

            def chain(bi):
                if prev_grp[1] is None:
                    prev_grp[1] = bi
                    if prev_grp[0] is not None:
                        add_dep_helper(bi.ins, prev_grp[0].ins, sync=False,
                                       reason="PE group-order chain")

            def mm1(ci):
                _, S = CHUNKS16[ci]
                xv = xt_chunk[ci]
                tiles = []
                for fi in range(FT):
                    hh, fw0 = fi // 8, (fi % 8) * P
                    group_start()
                    ph = psh.tile([P, 512], F32, tag="psh")
                    for kt in range(KT1):
                        chain(nc.tensor.matmul(
                            ph[:, :S],
                            w1_v[:, hh, kt, fw0 : fw0 + P],
                            xv[:, kt, 0:S],
                            start=(kt == 0),
                            stop=(kt == KT1 - 1),
                        ))
                    ht = hpool.tile([P, S], F16, tag="h")
                    nc.scalar.activation(
                        ht[:],
                        ph[:, :S],
                        mybir.ActivationFunctionType.Relu,
                        bias=bc_t[:, fi : fi + 1],
                    )
                    tiles.append(ht)
                h_tiles[ci] = tiles

            def mm2(ci):
                c0, S = CHUNKS16[ci]
                tiles = h_tiles.pop(ci)
                for mi in range(S // P):
                    group_start()
                    py = psy.tile([P, D], F32, tag="psy")
                    for kt in range(KT2):
                        chain(nc.tensor.matmul(
                            py[:],
                            tiles[kt][:, mi * P : (mi + 1) * P],
                            w2_t[:, kt * D : (kt + 1) * D],
                            start=(kt == 0),
                            stop=(kt == KT2 - 1),
                        ))
                    yt = ypool.tile([P, D], F16, tag="y")
                    ct = c0 // P + mi
                    nc.vector.tensor_scalar_mul(
                        yt[:], py[:], bc_t[:, FT + ct : FT + ct + 1]
                    )
                    nc.gpsimd.dma_start(y_d[ct * P : (ct + 1) * P, :], yt[:])

            def mm1_8():
                # fp8 DoubleRow: contraction 256/instruction over (q, j=2, p).
                for fi in range(FT):
                    hh, fw0 = fi // 8, (fi % 8) * P
                    group_start()
                    ph = psh.tile([P, 512], F32, tag="psh")
                    for q in range(2):
                        chain(nc.tensor.matmul(
                            ph[:, :C8],
                            w18_v[:, hh, q, :, fw0 : fw0 + P],
                            x8_v[:, q, :, :],
                            start=(q == 0),
                            stop=(q == 1),
                            perf_mode=mybir.MatmulPerfMode.DoubleRow,
                        ))
                    nc.scalar.activation(
                        h8_t[:, fi * C8 : (fi + 1) * C8],
                        ph[:, :C8],
                        mybir.ActivationFunctionType.Relu,
                        bias=bc_t[:, fi : fi + 1],
                        scale=H_SCALE,
                    )

            def mm2_8():
                for mi in range(C8 // P):
                    group_start()
                    py = psy.tile([P, D], F32, tag="psy")
                    for g in range(8):
                        chain(nc.tensor.matmul(
                            py[:],
                            h8_v[:, g, :, mi * P : (mi + 1) * P],
                            w28_v[:, g, :, :],
                            start=(g == 0),
                            stop=(g == 7),
                            perf_mode=mybir.MatmulPerfMode.DoubleRow,
                        ))
                    yt = ypool.tile([P, D], F16, tag="y")
                    ct = C16 // P + mi
                    nc.vector.tensor_scalar_mul(
                        yt[:], py[:], bc_t[:, FT + ct : FT + ct + 1]
                    )
                    nc.gpsimd.dma_start(y_d[ct * P : (ct + 1) * P, :], yt[:])

            # fp8 phase first (smallest critical DMA footprint), then the
            # fp16 chunks software-pipelined: mm1(ci) then mm2(ci-1). mm2_8
            # sits after mm1(c0) so the h8 relu latency and w2_8 stream hide.
            n16 = len(CHUNKS16)
            mm1_8()
            mm1(0)
            mm2_8()
            for ci in range(1, n16):
                mm1(ci)
                mm2(ci - 1)
            mm2(n16 - 1)

            # Keep the PE (and the HAM clock) busy while the DMA drain and the
            # fixed semaphore-clear epilogue run; these hide behind the drain.
            for _ in range(N_TAILJUNK):
                group_start()
                chain(nc.tensor.matmul(
                    wps[:], warm[:, 0:P], warm[:], start=True, stop=True
                ))

    nc.compile()
    _BUILD_CACHE["nc"] = nc
    return nc


def _q8(a, scale):
    return np.clip(a * scale, -240.0, 240.0).astype(NP_F8)


def _swz(a, kt):
    """[kt*128, X] row-major -> SBUF layout [128, kt*X] (one run/partition)."""
    return np.ascontiguousarray(
        a.reshape(kt, P, a.shape[1]).transpose(1, 0, 2).reshape(P, kt * a.shape[1])
    )


def kernel(x, Wr, br, W1, b1, W2, b2):
    x = np.ascontiguousarray(np.asarray(x, np.float32))
    Wr = np.asarray(Wr, np.float32)
    br = np.asarray(br, np.float32)
    W1 = np.ascontiguousarray(np.asarray(W1, np.float32))
    b1 = np.ascontiguousarray(np.asarray(b1, np.float32))
    W2 = np.ascontiguousarray(np.asarray(W2, np.float32))
    b2 = np.asarray(b2, np.float32)

    xf = x.reshape(N, D)

    # ---- host router: softmax -> top-2 -> combine weights ----
    logits = xf @ Wr + br
    m = logits.max(axis=-1, keepdims=True)
    p = np.exp(logits - m, dtype=np.float32)
    p /= p.sum(axis=-1, keepdims=True)
    idx = np.argpartition(-p, TOPK - 1, axis=-1)[:, :TOPK]
    cw = np.zeros((N, E), np.float32)
    np.put_along_axis(cw, idx, np.take_along_axis(p, idx, axis=-1), axis=-1)

    # Per expert, order tokens by ascending combine weight: the C8 smallest
    # run in fp8 (their quantization error is scaled by the small cw), the
    # next C16 in fp16, and the highest-cw overflow tail (~1.5% of pairs)
    # is computed exactly in fp32 on the host during the combine.
    tok, cwk = [], []
    for e in range(E):
        te = np.nonzero(cw[:, e] > 0)[0]
        order = np.argsort(cw[te, e], kind="stable")
        tok.append(te[order])
        cwk.append(cw[te[order], e])

    in_maps = []
    for e in range(E):
        te, ce = tok[e], len(tok[e])
        t8 = te[:C8]
        t16 = te[C8 : min(ce, C)]
        n16 = len(t16)
        xt_full = np.zeros((D, C16), np.float16)
        xt_full[:, :n16] = xf[t16].T
        # chunk-packed swizzle: [p, (chunk kt c)]
        xtp = np.concatenate(
            [_swz(xt_full[:, c0 : c0 + S], KT1) for c0, S in CHUNKS16], axis=1
        )
        x8p = _swz(_q8(xf[t8].T, SX), KT1)
        w1f = W1[e].astype(np.float16)
        w1p = np.concatenate(
            [_swz(w1f[:, 0:1024], KT1), _swz(w1f[:, 1024:2048], KT1)], axis=1
        )
        w18f = _q8(W1[e], SW1)
        w18p = np.concatenate(
            [_swz(w18f[:, 0:1024], KT1), _swz(w18f[:, 1024:2048], KT1)], axis=1
        )
        w2p = _swz(W2[e].astype(np.float16), KT2)
        w28p = _swz(_q8(W2[e], SW2), KT2)
        bcp = np.zeros((P, FT + CT), np.float32)
        bcp[:, :FT] = b1[e].reshape(FT, P).T
        cwe = np.zeros((C,), np.float32)
        cwe[:C8] = cwk[e][:C8] * CW8_SCALE
        cwe[C8 : C8 + n16] = cwk[e][C8 : C8 + n16]
        # device token order is [fp16 block | fp8 block] in the y tensor
        cwdev = np.concatenate(
            [cwe[C8 : C8 + n16], np.zeros(C16 - n16, np.float32), cwe[:C8]]
        )
        bcp[:, FT:] = cwdev.reshape(CT, P).T
        in_maps.append(
            {
                "xt": xtp,
                "x8": x8p,
                "w1": w1p,
                "w2": w2p,
                "w18": w18p,
                "w28": w28p,
                "bc": bcp,
            }
        )

    nc = _build()
    trace = bool(os.environ.get("BASS_MOE_TRACE"))
    try:
        res = run_bass_kernel_spmd(
            nc,
            in_maps,
            core_ids=list(range(N_CORES)),
            trace=trace,
            trace_cores=list(range(N_CORES)) if trace else None,
        )
    except Exception:
        if not trace:
            raise
        trace = False
        res = run_bass_kernel_spmd(nc, in_maps, core_ids=list(range(N_CORES)))
    if trace and res.exec_time_ns is not None:
        print(f"HW exec time: {res.exec_time_ns} ns")
        print(f"mean exec time: {res.mean_exec_time_ns} ns")
        if res.instructions_and_trace is not None:
            print(f"trace: {res.instructions_and_trace[1]}")

    # ---- host combine: scatter-add expert outputs + cw-weighted b2 ----
    out = cw @ b2
    for e in range(E):
        te, ce = tok[e], len(tok[e])
        y = res.results[e]["y"].astype(np.float32)
        n16 = min(ce, C) - C8
        out[te[C8 : C8 + n16]] += y[:n16]
        out[te[:C8]] += y[C16:]
        th = te[C:]  # capacity-overflow tail: exact fp32 on host
        if len(th):
            yh = np.maximum(xf[th] @ W1[e] + b1[e], 0.0) @ W2[e]
            out[th] += cw[th, e][:, None] * yh
    return out.reshape(B, T, D)


# revision 22
# speedup vs baseline: 1.0349x; 1.0349x over previous
"""MoE layer (B=8,T=1024,D=512,F=2048,E=8,top-2) on 8 NeuronCores.

Strategy (expert parallel, per the sharding hint):
- Host computes the router (logits -> softmax -> top-2 -> combine weights);
  tokens are gathered per expert and dispatched to the core owning that
  expert (the "all-to-all by routing assignment" is the host gather).
- Core e runs the expert-e FFN over its gathered tokens, split in two
  precision classes by combine weight: the C8 tokens with the SMALLEST
  combine weights run both matmuls in fp8-e4m3 DoubleRow mode (~1.45x PE
  rate; their output error is scaled by the small cw, keeping total
  rel-err under the 2e-2 gate), the remaining C16 tokens run in fp16.
- All device tensors are shipped pre-swizzled into SBUF layout ([128
  partitions, free]) so every DMA is one long contiguous run per
  partition: the DMA rings here are packet-rate limited, and the naive
  [(k p) f] strided loads cost 4x the packets for the same bytes.
- Host scatter-adds the per-expert outputs back (plus the cw-weighted b2
  rank-1 term); the ~1.5% capacity-overflow tail of highest-cw tokens is
  computed exactly in fp32 on the host, as in the baseline.
"""

import os
import numpy as np
import ml_dtypes

import concourse.bass as bass
from bass_rust import add_dep_helper
import concourse.tile as tile
from concourse import bacc, mybir
from concourse.bass_utils import run_bass_kernel_spmd

F32 = mybir.dt.float32
F16 = mybir.dt.float16
F8 = mybir.dt.float8e4
NP_F8 = ml_dtypes.float8_e4m3  # TRN e4m3 (max normal 240)

B, T, D, F, E, TOPK = 8, 1024, 512, 2048, 8, 2
N = B * T
P = 128
N_CORES = 8
KT1 = D // P    # 4  k-tiles for x @ W1
KT2 = F // P    # 16 k-tiles for h @ W2
FT = F // P     # 16 f-tiles of hT

C = 2048        # device token capacity per expert (= N*TOPK / E exactly)
C8 = 384        # lowest-cw tokens per expert in fp8 DoubleRow
C16 = C - C8    # fp16 tokens per expert
CT = C // P

# fp8 scales (powers of two, folded exactly):
#   x8 = q8(x*16), w1_8 = q8(W1*4096)  -> psum = (x@W1) * 2^16
#   h8 = relu(psum * 2^-16 + b1)       -> h in natural units, fp8
#   w2_8 = q8(W2*8192)                 -> psum = y * 2^13
#   y = psum * (cw * 2^-13)            -> cw pre-scaled on host
SX, SW1, SW2 = 16.0, 4096.0, 8192.0
H_SCALE = 1.0 / 65536.0
CW8_SCALE = 1.0 / 8192.0

CHUNKS16 = [(0, 384), (384, 512), (896, 512), (1408, 256)]  # sums to C16
XT_OFF = []
_o = 0
for _c0, _s in CHUNKS16:
    XT_OFF.append(_o)
    _o += KT1 * _s
XT_COLS = _o            # 4*C16
N_WARM = 8      # PE clock-ramp matmuls before real data lands
N_TAILJUNK = 12  # post-work matmuls keeping HAM at 8/8 through the epilogue

_BUILD_CACHE = {}


def _build():
    if "nc" in _BUILD_CACHE:
        return _BUILD_CACHE["nc"]
    nc = bacc.Bacc()

    # All inputs are host-preswizzled to SBUF layout [128, free].
    xt_d = nc.dram_tensor("xt", [P, XT_COLS], F16, kind="ExternalInput")
    x8_d = nc.dram_tensor("x8", [P, 4 * C8], F8, kind="ExternalInput")
    w1_d = nc.dram_tensor("w1", [P, KT1 * F], F16, kind="ExternalInput")
    w2_d = nc.dram_tensor("w2", [P, KT2 * D], F16, kind="ExternalInput")
    w18_d = nc.dram_tensor("w18", [P, KT1 * F], F8, kind="ExternalInput")
    w28_d = nc.dram_tensor("w28", [P, KT2 * D], F8, kind="ExternalInput")
    # b1 in cols 0:FT, cw in cols FT:FT+CT (cw cols for fp8 tokens pre-scaled)
    bc_d = nc.dram_tensor("bc", [P, FT + CT], F32, kind="ExternalInput")
    y_d = nc.dram_tensor("y", [C, D], F16, kind="ExternalOutput")

    with tile.TileContext(nc) as tc:
        with (
            tc.tile_pool(name="weights", bufs=1) as wpool,
            tc.tile_pool(name="xt", bufs=1) as xpool,
            tc.tile_pool(name="h", bufs=2 * FT + 1) as hpool,
            tc.tile_pool(name="y", bufs=6) as ypool,
            tc.tile_pool(name="psh", bufs=3, space="PSUM") as psh,
            tc.tile_pool(name="psy", bufs=3, space="PSUM") as psy,
        ):
            # ---- tiles (layouts match the host swizzle exactly) ----
            # w1:  [p, h(2), kt(4), fw(1024)]  h = fi//8, fw = (fi%8)*128 + ...
            w1_t = wpool.tile([P, KT1 * F], F16, tag="w1")
            w1_v = w1_t[:].rearrange("p (h kt fw) -> p h kt fw", h=2, kt=KT1)
            # w18: [p, h(2), q(2), j(2), fw(1024)]
            w18_t = wpool.tile([P, KT1 * F], F8, tag="w18")
            w18_v = w18_t[:].rearrange(
                "p (h q j fw) -> p h q j fw", h=2, q=2, j=2
            )
            # w2:  [p, kt(16), d(512)]
            w2_t = wpool.tile([P, KT2 * D], F16, tag="w2")
            # w28: [p, g(8), j(2), d(512)]
            w28_t = wpool.tile([P, KT2 * D], F8, tag="w28")
            w28_v = w28_t[:].rearrange("p (g j d) -> p g j d", g=8, j=2)
            bc_t = wpool.tile([P, FT + CT], F32, tag="bc")
            # xt: per chunk [p, kt(4), c(S)] packed back to back
            xt_t = xpool.tile([P, XT_COLS], F16, tag="xt")
            xt_chunk = [
                xt_t[:, XT_OFF[ci] : XT_OFF[ci] + KT1 * S].rearrange(
                    "p (kt c) -> p kt c", kt=KT1
                )
                for ci, (_, S) in enumerate(CHUNKS16)
            ]
            # x8: [p, q(2), j(2), c(C8)]
            x8_t = xpool.tile([P, 4 * C8], F8, tag="x8")
            x8_v = x8_t[:].rearrange("p (q j c) -> p q j c", q=2, j=2)
            h8_t = wpool.tile([P, FT * C8], F8, tag="h8")
            h8_v = h8_t[:].rearrange("p (g j c) -> p g j c", g=8, j=2)

            # PE warm-up: junk matmuls on a zeroed tile while the input DMAs
            # stream, so the HAM clock-gate reaches 8/8 before real work.
            warm = wpool.tile([P, 512], F16, tag="warm")
            nc.gpsimd.memset(warm[:], 0.0)
            wps = psy.tile([P, 512], F32, tag="psy")
            for _ in range(N_WARM):
                nc.tensor.matmul(wps[:], warm[:, 0:P], warm[:], start=True, stop=True)

            # ---- DMA plan ----
            # fp8 phase runs first: its critical inputs (x8 + w18 low half)
            # land while the warm-up ramps the clock. Every DMA below is one
            # contiguous <=4KB run per partition.
            nc.sync.dma_start(x8_t[:], x8_d[:])
            nc.sync.dma_start(w18_t[:, 0 : KT1 * F // 2], w18_d[:, 0 : KT1 * F // 2])
            nc.sync.dma_start(w18_t[:, KT1 * F // 2 :], w18_d[:, KT1 * F // 2 :])
            s0, e0 = XT_OFF[0], XT_OFF[0] + KT1 * CHUNKS16[0][1]
            nc.sync.dma_start(xt_t[:, s0:e0], xt_d[:, s0:e0])
            # w1 high half (fi8-15) in two 4KB pieces
            nc.sync.dma_start(w1_t[:, 4096:6144], w1_d[:, 4096:6144])
            nc.sync.dma_start(w1_t[:, 6144:8192], w1_d[:, 6144:8192])
            for ci in range(1, len(CHUNKS16)):
                s, e = XT_OFF[ci], XT_OFF[ci] + KT1 * CHUNKS16[ci][1]
                nc.sync.dma_start(xt_t[:, s:e], xt_d[:, s:e])

            nc.gpsimd.dma_start(bc_t[:], bc_d[:])
            # w1 low half (fi0-7) in two 4KB pieces
            nc.gpsimd.dma_start(w1_t[:, 0:2048], w1_d[:, 0:2048])
            nc.gpsimd.dma_start(w1_t[:, 2048:4096], w1_d[:, 2048:4096])
            # w28 (8KB) in two pieces, then w2 (16KB) in four pieces
            nc.gpsimd.dma_start(w28_t[:, 0:4096], w28_d[:, 0:4096])
            nc.gpsimd.dma_start(w28_t[:, 4096:8192], w28_d[:, 4096:8192])
            for k in range(4):
                nc.gpsimd.dma_start(
                    w2_t[:, k * 2048 : (k + 1) * 2048],
                    w2_d[:, k * 2048 : (k + 1) * 2048],
                )

            # ---- PE group-order chain (pin issue order to program order) ----
            h_tiles = {}
            prev_grp = [None, None]

            def group_start():
                prev_grp[0], prev_grp[1] = prev_grp[1], None

            def chain(bi):
                if prev_grp[1] is None:
                    prev_grp[1] = bi
                    if prev_grp[0] is not None:
                        add_dep_helper(bi.ins, prev_grp[0].ins, sync=False,
                                       reason="PE group-order chain")

            def mm1(ci):
                _, S = CHUNKS16[ci]
                xv = xt_chunk[ci]
                tiles = []
                for fi in range(FT):
                    hh, fw0 = fi // 8, (fi % 8) * P
                    group_start()
                    ph = psh.tile([P, 512], F32, tag="psh")
                    for kt in range(KT1):
                        chain(nc.tensor.matmul(
                            ph[:, :S],
                            w1_v[:, hh, kt, fw0 : fw0 + P],
                            xv[:, kt, 0:S],
                            start=(kt == 0),
                            stop=(kt == KT1 - 1),
                        ))
                    ht = hpool.tile([P, S], F16, tag="h")
                    nc.scalar.activation(
                        ht[:],
                        ph[:, :S],
                        mybir.ActivationFunctionType.Relu,
                        bias=bc_t[:, fi : fi + 1],
                    )
                    tiles.append(ht)
                h_tiles[ci] = tiles

            def mm2(ci):
                c0, S = CHUNKS16[ci]
                tiles = h_tiles.pop(ci)
                for mi in range(S // P):
                    group_start()
                    py = psy.tile([P, D], F32, tag="psy")
                    for kt in range(KT2):
                        chain(nc.tensor.matmul(
                            py[:],
                            tiles[kt][:, mi * P : (mi + 1) * P],
                            w2_t[:, kt * D : (kt + 1) * D],
                            start=(kt == 0),
                            stop=(kt == KT2 - 1),
                        ))
                    yt = ypool.tile([P, D], F16, tag="y")
                    ct = c0 // P + mi
                    nc.vector.tensor_scalar_mul(
                        yt[:], py[:], bc_t[:, FT + ct : FT + ct + 1]
                    )
                    nc.gpsimd.dma_start(y_d[ct * P : (ct + 1) * P, :], yt[:])

            def mm1_8():
                # fp8 DoubleRow: contraction 256/instruction over (q, j=2, p).
                for fi in range(FT):
                    hh, fw0 = fi // 8, (fi % 8) * P
                    group_start()
                    ph = psh.tile([P, 512], F32, tag="psh")
                    for q in range(2):
                        chain(nc.tensor.matmul(
                            ph[:, :C8],
                            w18_v[:, hh, q, :, fw0 : fw0 + P],
                            x8_v[:, q, :, :],
                            start=(q == 0),
                            stop=(q == 1),
                            perf_mode=mybir.MatmulPerfMode.DoubleRow,
                        ))
                    nc.scalar.activation(
                        h8_t[:, fi * C8 : (fi + 1) * C8],
                        ph[:, :C8],
                        mybir.ActivationFunctionType.Relu,
                        bias=bc_t[:, fi : fi + 1],
                        scale=H_SCALE,
                    )

            def mm2_8():
                for mi in range(C8 // P):
                    group_start()
                    py = psy.tile([P, D], F32, tag="psy")
                    for g in range(8):
                        chain(nc.tensor.matmul(
                            py[:],
                            h8_v[:, g, :, mi * P : (mi + 1) * P],
                            w28_v[:, g, :, :],
                            start=(g == 0),
                            stop=(g == 7),
                            perf_mode=mybir.MatmulPerfMode.DoubleRow,
                        ))
                    yt = ypool.tile([P, D], F16, tag="y")
                    ct = C16 // P + mi
                    nc.vector.tensor_scalar_mul(
                        yt[:], py[:], bc_t[:, FT + ct : FT + ct + 1]
                    )
                    nc.gpsimd.dma_start(y_d[ct * P : (ct + 1) * P, :], yt[:])

            # fp8 phase first (smallest critical DMA footprint), then the
            # fp16 chunks software-pipelined: mm1(ci) then mm2(ci-1). mm2_8
            # sits after mm1(c0) so the h8 relu latency and w2_8 stream hide.
            n16 = len(CHUNKS16)
            mm1_8()
            mm1(0)
            mm2_8()
            for ci in range(1, n16):
                mm1(ci)
                mm2(ci - 1)
            mm2(n16 - 1)

            # Keep the PE (and the HAM clock) busy while the DMA drain and the
            # fixed semaphore-clear epilogue run; these hide behind the drain.
            for _ in range(N_TAILJUNK):
                group_start()
                chain(nc.tensor.matmul(
                    wps[:], warm[:, 0:P], warm[:], start=True, stop=True
                ))

    nc.compile()
    _BUILD_CACHE["nc"] = nc
    return nc


def _q8(a, scale):
    return np.clip(a * scale, -240.0, 240.0).astype(NP_F8)


def _swz(a, kt):
    """[kt*128, X] row-major -> SBUF layout [128, kt*X] (one run/partition)."""
    return np.ascontiguousarray(
        a.reshape(kt, P, a.shape[1]).transpose(1, 0, 2).reshape(P, kt * a.shape[1])
    )


def kernel(x, Wr, br, W1, b1, W2, b2):
    x = np.ascontiguousarray(np.asarray(x, np.float32))
    Wr = np.asarray(Wr, np.float32)
    br = np.asarray(br, np.float32)
    W1 = np.ascontiguousarray(np.asarray(W1, np.float32))
    b1 = np.ascontiguousarray(np.asarray(b1, np.float32))
    W2 = np.ascontiguousarray(np.asarray(W2, np.float32))
    b2 = np.asarray(b2, np.float32)

    xf = x.reshape(N, D)

    # ---- host router: softmax -> top-2 -> combine weights ----
    logits = xf @ Wr + br
    m = logits.max(axis=-1, keepdims=True)
    p = np.exp(logits - m, dtype=np.float32)
    p /= p.sum(axis=-1, keepdims=True)
    idx = np.argpartition(-p, TOPK - 1, axis=-1)[:, :TOPK]
    cw = np.zeros((N, E), np.float32)
    np.put_along_axis(cw, idx, np.take_along_axis(p, idx, axis=-1), axis=-1)

    # Per expert, order tokens by ascending combine weight: the C8 smallest
    # run in fp8 (their quantization error is scaled by the small cw), the
    # next C16 in fp16, and the highest-cw overflow tail (~1.5% of pairs)
    # is computed exactly in fp32 on the host during the combine.
    tok, cwk = [], []
    for e in range(E):
        te = np.nonzero(cw[:, e] > 0)[0]
        order = np.argsort(cw[te, e], kind="stable")
        tok.append(te[order])
        cwk.append(cw[te[order], e])

    in_maps = []
    for e in range(E):
        te, ce = tok[e], len(tok[e])
        t8 = te[:C8]
        t16 = te[C8 : min(ce, C)]
        n16 = len(t16)
        xt_full = np.zeros((D, C16), np.float16)
        xt_full[:, :n16] = xf[t16].T
        # chunk-packed swizzle: [p, (chunk kt c)]
        xtp = np.concatenate(
            [_swz(xt_full[:, c0 : c0 + S], KT1) for c0, S in CHUNKS16], axis=1
        )
        x8p = _swz(_q8(xf[t8].T, SX), KT1)
        w1f = W1[e].astype(np.float16)
        w1p = np.concatenate(
            [_swz(w1f[:, 0:1024], KT1), _swz(w1f[:, 1024:2048], KT1)], axis=1
        )
        w18f = _q8(W1[e], SW1)
        w18p = np.concatenate(
            [_swz(w18f[:, 0:1024], KT1), _swz(w18f[:, 1024:2048], KT1)], axis=1
        )
        w2p = _swz(W2[e].astype(np.float16), KT2)
        w28p = _swz(_q8(W2[e], SW2), KT2)
        bcp = np.zeros((P, FT + CT), np.float32)
        bcp[:, :FT] = b1[e].reshape(FT, P).T
        cwe = np.zeros((C,), np.float32)
        cwe[:C8] = cwk[e][:C8] * CW8_SCALE
        cwe[C8 : C8 + n16] = cwk[e][C8 : C8 + n16]
        # device token order is [fp16 block | fp8 block] in the y tensor
        cwdev = np.concatenate(
            [cwe[C8 : C8 + n16], np.zeros(C16 - n16, np.float32), cwe[:C8]]
        )
        bcp[:, FT:] = cwdev.reshape(CT, P).T
        in_maps.append(
            {
                "xt": xtp,
                "x8": x8p,
                "w1": w1p,
                "w2": w2p,
                "w18": w18p,
                "w28": w28p,
                "bc": bcp,
            }
        )

    nc = _build()
    trace = bool(os.environ.get("BASS_MOE_TRACE"))
    try:
        res = run_bass_kernel_spmd(
            nc,
            in_maps,
            core_ids=list(range(N_CORES)),
            trace=trace,
            trace_cores=list(range(N_CORES)) if trace else None,
        )
    except Exception:
        if not trace:
            raise
        trace = False
        res = run_bass_kernel_spmd(nc, in_maps, core_ids=list(range(N_CORES)))
    if trace and res.exec_time_ns is not None:
        print(f"HW exec time: {res.exec_time_ns} ns")
        print(f"mean exec time: {res.mean_exec_time_ns} ns")
        if res.instructions_and_trace is not None:
            print(f"trace: {res.instructions_and_trace[1]}")

    # ---- host combine: scatter-add expert outputs + cw-weighted b2 ----
    out = cw @ b2
    for e in range(E):
        te, ce = tok[e], len(tok[e])
        y = res.results[e]["y"].astype(np.float32)
        n16 = min(ce, C) - C8
        out[te[C8 : C8 + n16]] += y[:n16]
        out[te[:C8]] += y[C16:]
        th = te[C:]  # capacity-overflow tail: exact fp32 on host
        if len(th):
            yh = np.maximum(xf[th] @ W1[e] + b1[e], 0.0) @ W2[e]
            out[th] += cw[th, e][:, None] * yh
    return out.reshape(B, T, D)


# revision 23
# speedup vs baseline: 1.0463x; 1.0111x over previous
"""MoE layer (B=8,T=1024,D=512,F=2048,E=8,top-2) on 8 NeuronCores.

Strategy (expert parallel, per the sharding hint):
- Host computes the router (logits -> softmax -> top-2 -> combine weights);
  tokens are gathered per expert and dispatched to the core owning that
  expert (the "all-to-all by routing assignment" is the host gather).
- Core e runs the expert-e FFN over its gathered tokens, split in two
  precision classes by combine weight: the C8 tokens with the SMALLEST
  combine weights run both matmuls in fp8-e4m3 DoubleRow mode (~1.45x PE
  rate; their output error is scaled by the small cw, keeping total
  rel-err under the 2e-2 gate), the remaining C16 tokens run in fp16.
- All device tensors are shipped pre-swizzled into SBUF layout ([128
  partitions, free]) so every DMA is one long contiguous run per
  partition: the DMA rings here are packet-rate limited, and the naive
  [(k p) f] strided loads cost 4x the packets for the same bytes.
- Host scatter-adds the per-expert outputs back (plus the cw-weighted b2
  rank-1 term); the ~1.5% capacity-overflow tail of highest-cw tokens is
  computed exactly in fp32 on the host, as in the baseline.
"""

import os
import numpy as np
import ml_dtypes

import concourse.bass as bass
from bass_rust import add_dep_helper
import concourse.tile as tile
from concourse import bacc, mybir
from concourse.bass_utils import run_bass_kernel_spmd

F32 = mybir.dt.float32
F16 = mybir.dt.float16
F8 = mybir.dt.float8e4
NP_F8 = ml_dtypes.float8_e4m3  # TRN e4m3 (max normal 240)

B, T, D, F, E, TOPK = 8, 1024, 512, 2048, 8, 2
N = B * T
P = 128
N_CORES = 8
KT1 = D // P    # 4  k-tiles for x @ W1
KT2 = F // P    # 16 k-tiles for h @ W2
FT = F // P     # 16 f-tiles of hT

C = 2048        # device token capacity per expert (= N*TOPK / E exactly)
C8 = 384        # lowest-cw tokens per expert in fp8 DoubleRow
C16 = C - C8    # fp16 tokens per expert
CT = C // P

# fp8 scales (powers of two, folded exactly):
#   x8 = q8(x*16), w1_8 = q8(W1*4096)  -> psum = (x@W1) * 2^16
#   h8 = relu(psum * 2^-16 + b1)       -> h in natural units, fp8
#   w2_8 = q8(W2*8192)                 -> psum = y * 2^13
#   y = psum * (cw * 2^-13)            -> cw pre-scaled on host
SX, SW1, SW2 = 16.0, 4096.0, 8192.0
H_SCALE = 1.0 / 65536.0
CW8_SCALE = 1.0 / 8192.0

CHUNKS16 = [(0, 384), (384, 512), (896, 512), (1408, 256)]  # sums to C16
XT_OFF = []
_o = 0
for _c0, _s in CHUNKS16:
    XT_OFF.append(_o)
    _o += KT1 * _s
XT_COLS = _o            # 4*C16
N_WARM = 12     # PE clock-ramp matmuls before real data lands
N_TAILJUNK = 12  # post-work matmuls keeping HAM at 8/8 through the epilogue

_BUILD_CACHE = {}


def _build():
    if "nc" in _BUILD_CACHE:
        return _BUILD_CACHE["nc"]
    nc = bacc.Bacc()

    # All inputs are host-preswizzled to SBUF layout [128, free].
    xt_d = nc.dram_tensor("xt", [P, XT_COLS], F16, kind="ExternalInput")
    x8_d = nc.dram_tensor("x8", [P, 4 * C8], F8, kind="ExternalInput")
    w1_d = nc.dram_tensor("w1", [P, KT1 * F], F16, kind="ExternalInput")
    w2_d = nc.dram_tensor("w2", [P, KT2 * D], F16, kind="ExternalInput")
    w18_d = nc.dram_tensor("w18", [P, KT1 * F], F8, kind="ExternalInput")
    w28_d = nc.dram_tensor("w28", [P, KT2 * D], F8, kind="ExternalInput")
    # b1 in cols 0:FT, cw in cols FT:FT+CT (cw cols for fp8 tokens pre-scaled)
    bc_d = nc.dram_tensor("bc", [P, FT + CT], F32, kind="ExternalInput")
    y_d = nc.dram_tensor("y", [C, D], F16, kind="ExternalOutput")

    with tile.TileContext(nc) as tc:
        with (
            tc.tile_pool(name="weights", bufs=1) as wpool,
            tc.tile_pool(name="xt", bufs=1) as xpool,
            tc.tile_pool(name="h", bufs=2 * FT + 1) as hpool,
            tc.tile_pool(name="y", bufs=6) as ypool,
            tc.tile_pool(name="psh", bufs=3, space="PSUM") as psh,
            tc.tile_pool(name="psy", bufs=3, space="PSUM") as psy,
        ):
            # ---- tiles (layouts match the host swizzle exactly) ----
            # w1:  [p, h(2), kt(4), fw(1024)]  h = fi//8, fw = (fi%8)*128 + ...
            w1_t = wpool.tile([P, KT1 * F], F16, tag="w1")
            w1_v = w1_t[:].rearrange("p (h kt fw) -> p h kt fw", h=2, kt=KT1)
            # w18: [p, h(2), q(2), j(2), fw(1024)]
            w18_t = wpool.tile([P, KT1 * F], F8, tag="w18")
            w18_v = w18_t[:].rearrange(
                "p (h q j fw) -> p h q j fw", h=2, q=2, j=2
            )
            # w2:  [p, kt(16), d(512)]
            w2_t = wpool.tile([P, KT2 * D], F16, tag="w2")
            # w28: [p, g(8), j(2), d(512)]
            w28_t = wpool.tile([P, KT2 * D], F8, tag="w28")
            w28_v = w28_t[:].rearrange("p (g j d) -> p g j d", g=8, j=2)
            bc_t = wpool.tile([P, FT + CT], F32, tag="bc")
            # xt: per chunk [p, kt(4), c(S)] packed back to back
            xt_t = xpool.tile([P, XT_COLS], F16, tag="xt")
            xt_chunk = [
                xt_t[:, XT_OFF[ci] : XT_OFF[ci] + KT1 * S].rearrange(
                    "p (kt c) -> p kt c", kt=KT1
                )
                for ci, (_, S) in enumerate(CHUNKS16)
            ]
            # x8: [p, q(2), j(2), c(C8)]
            x8_t = xpool.tile([P, 4 * C8], F8, tag="x8")
            x8_v = x8_t[:].rearrange("p (q j c) -> p q j c", q=2, j=2)
            h8_t = wpool.tile([P, FT * C8], F8, tag="h8")
            h8_v = h8_t[:].rearrange("p (g j c) -> p g j c", g=8, j=2)

            # PE warm-up: junk matmuls on a zeroed tile while the input DMAs
            # stream, so the HAM clock-gate reaches 8/8 before real work.
            warm = wpool.tile([P, 512], F16, tag="warm")
            nc.gpsimd.memset(warm[:], 0.0)
            wps = psy.tile([P, 512], F32, tag="psy")
            for _ in range(N_WARM):
                nc.tensor.matmul(wps[:], warm[:, 0:P], warm[:], start=True, stop=True)

            # ---- DMA plan ----
            # fp8 phase runs first: its critical inputs (x8 + w18 low half)
            # land while the warm-up ramps the clock. Every DMA below is one
            # contiguous <=4KB run per partition.
            nc.sync.dma_start(x8_t[:], x8_d[:])
            nc.sync.dma_start(w18_t[:, 0 : KT1 * F // 2], w18_d[:, 0 : KT1 * F // 2])
            nc.sync.dma_start(w18_t[:, KT1 * F // 2 :], w18_d[:, KT1 * F // 2 :])
            s0, e0 = XT_OFF[0], XT_OFF[0] + KT1 * CHUNKS16[0][1]
            nc.sync.dma_start(xt_t[:, s0:e0], xt_d[:, s0:e0])
            # w1 high half (fi8-15) in two 4KB pieces
            nc.sync.dma_start(w1_t[:, 4096:6144], w1_d[:, 4096:6144])
            nc.sync.dma_start(w1_t[:, 6144:8192], w1_d[:, 6144:8192])
            for ci in range(1, len(CHUNKS16)):
                s, e = XT_OFF[ci], XT_OFF[ci] + KT1 * CHUNKS16[ci][1]
                nc.sync.dma_start(xt_t[:, s:e], xt_d[:, s:e])

            nc.gpsimd.dma_start(bc_t[:], bc_d[:])
            # w1 low half (fi0-7) in two 4KB pieces
            nc.gpsimd.dma_start(w1_t[:, 0:2048], w1_d[:, 0:2048])
            nc.gpsimd.dma_start(w1_t[:, 2048:4096], w1_d[:, 2048:4096])
            # w28 (8KB) in two pieces, then w2 (16KB) in four pieces
            nc.gpsimd.dma_start(w28_t[:, 0:4096], w28_d[:, 0:4096])
            nc.gpsimd.dma_start(w28_t[:, 4096:8192], w28_d[:, 4096:8192])
            for k in range(4):
                nc.gpsimd.dma_start(
                    w2_t[:, k * 2048 : (k + 1) * 2048],
                    w2_d[:, k * 2048 : (k + 1) * 2048],
                )

            # ---- PE group-order chain (pin issue order to program order) ----
            h_tiles = {}
            prev_grp = [None, None]

            def group_start():
                prev_grp[0], prev_grp[1] = prev_grp[1], None

            def chain(bi):
                if prev_grp[1] is None:
                    prev_grp[1] = bi
                    if prev_grp[0] is not None:
                        add_dep_helper(bi.ins, prev_grp[0].ins, sync=False,
                                       reason="PE group-order chain")

            def mm1(ci):
                _, S = CHUNKS16[ci]
                xv = xt_chunk[ci]
                tiles = []
                for fi in range(FT):
                    hh, fw0 = fi // 8, (fi % 8) * P
                    group_start()
                    ph = psh.tile([P, 512], F32, tag="psh")
                    for kt in range(KT1):
                        chain(nc.tensor.matmul(
                            ph[:, :S],
                            w1_v[:, hh, kt, fw0 : fw0 + P],
                            xv[:, kt, 0:S],
                            start=(kt == 0),
                            stop=(kt == KT1 - 1),
                        ))
                    ht = hpool.tile([P, S], F16, tag="h")
                    nc.scalar.activation(
                        ht[:],
                        ph[:, :S],
                        mybir.ActivationFunctionType.Relu,
                        bias=bc_t[:, fi : fi + 1],
                    )
                    tiles.append(ht)
                h_tiles[ci] = tiles

            def mm2(ci):
                c0, S = CHUNKS16[ci]
                tiles = h_tiles.pop(ci)
                for mi in range(S // P):
                    group_start()
                    py = psy.tile([P, D], F32, tag="psy")
                    for kt in range(KT2):
                        chain(nc.tensor.matmul(
                            py[:],
                            tiles[kt][:, mi * P : (mi + 1) * P],
                            w2_t[:, kt * D : (kt + 1) * D],
                            start=(kt == 0),
                            stop=(kt == KT2 - 1),
                        ))
                    yt = ypool.tile([P, D], F16, tag="y")
                    ct = c0 // P + mi
                    nc.vector.tensor_scalar_mul(
                        yt[:], py[:], bc_t[:, FT + ct : FT + ct + 1]
                    )
                    nc.gpsimd.dma_start(y_d[ct * P : (ct + 1) * P, :], yt[:])

            def mm1_8():
                # fp8 DoubleRow: contraction 256/instruction over (q, j=2, p).
                for fi in range(FT):
                    hh, fw0 = fi // 8, (fi % 8) * P
                    group_start()
                    ph = psh.tile([P, 512], F32, tag="psh")
                    for q in range(2):
                        chain(nc.tensor.matmul(
                            ph[:, :C8],
                            w18_v[:, hh, q, :, fw0 : fw0 + P],
                            x8_v[:, q, :, :],
                            start=(q == 0),
                            stop=(q == 1),
                            perf_mode=mybir.MatmulPerfMode.DoubleRow,
                        ))
                    nc.scalar.activation(
                        h8_t[:, fi * C8 : (fi + 1) * C8],
                        ph[:, :C8],
                        mybir.ActivationFunctionType.Relu,
                        bias=bc_t[:, fi : fi + 1],
                        scale=H_SCALE,
                    )

            def mm2_8():
                for mi in range(C8 // P):
                    group_start()
                    py = psy.tile([P, D], F32, tag="psy")
                    for g in range(8):
                        chain(nc.tensor.matmul(
                            py[:],
                            h8_v[:, g, :, mi * P : (mi + 1) * P],
                            w28_v[:, g, :, :],
                            start=(g == 0),
                            stop=(g == 7),
                            perf_mode=mybir.MatmulPerfMode.DoubleRow,
                        ))
                    yt = ypool.tile([P, D], F16, tag="y")
                    ct = C16 // P + mi
                    nc.vector.tensor_scalar_mul(
                        yt[:], py[:], bc_t[:, FT + ct : FT + ct + 1]
                    )
                    nc.gpsimd.dma_start(y_d[ct * P : (ct + 1) * P, :], yt[:])

            # fp8 phase first (smallest critical DMA footprint), then the
            # fp16 chunks software-pipelined: mm1(ci) then mm2(ci-1). mm2_8
            # sits after mm1(c0) so the h8 relu latency and w2_8 stream hide.
            n16 = len(CHUNKS16)
            mm1_8()
            mm1(0)
            mm2_8()
            for ci in range(1, n16):
                mm1(ci)
                mm2(ci - 1)
            mm2(n16 - 1)

            # Keep the PE (and the HAM clock) busy while the DMA drain and the
            # fixed semaphore-clear epilogue run; these hide behind the drain.
            for _ in range(N_TAILJUNK):
                group_start()
                chain(nc.tensor.matmul(
                    wps[:], warm[:, 0:P], warm[:], start=True, stop=True
                ))

    nc.compile()
    _BUILD_CACHE["nc"] = nc
    return nc


def _q8(a, scale):
    return np.clip(a * scale, -240.0, 240.0).astype(NP_F8)


def _swz(a, kt):
    """[kt*128, X] row-major -> SBUF layout [128, kt*X] (one run/partition)."""
    return np.ascontiguousarray(
        a.reshape(kt, P, a.shape[1]).transpose(1, 0, 2).reshape(P, kt * a.shape[1])
    )


def kernel(x, Wr, br, W1, b1, W2, b2):
    x = np.ascontiguousarray(np.asarray(x, np.float32))
    Wr = np.asarray(Wr, np.float32)
    br = np.asarray(br, np.float32)
    W1 = np.ascontiguousarray(np.asarray(W1, np.float32))
    b1 = np.ascontiguousarray(np.asarray(b1, np.float32))
    W2 = np.ascontiguousarray(np.asarray(W2, np.float32))
    b2 = np.asarray(b2, np.float32)

    xf = x.reshape(N, D)

    # ---- host router: softmax -> top-2 -> combine weights ----
    logits = xf @ Wr + br
    m = logits.max(axis=-1, keepdims=True)
    p = np.exp(logits - m, dtype=np.float32)
    p /= p.sum(axis=-1, keepdims=True)
    idx = np.argpartition(-p, TOPK - 1, axis=-1)[:, :TOPK]
    cw = np.zeros((N, E), np.float32)
    np.put_along_axis(cw, idx, np.take_along_axis(p, idx, axis=-1), axis=-1)

    # Per expert, order tokens by ascending combine weight: the C8 smallest
    # run in fp8 (their quantization error is scaled by the small cw), the
    # next C16 in fp16, and the highest-cw overflow tail (~1.5% of pairs)
    # is computed exactly in fp32 on the host during the combine.
    tok, cwk = [], []
    for e in range(E):
        te = np.nonzero(cw[:, e] > 0)[0]
        order = np.argsort(cw[te, e], kind="stable")
        tok.append(te[order])
        cwk.append(cw[te[order], e])

    in_maps = []
    for e in range(E):
        te, ce = tok[e], len(tok[e])
        t8 = te[:C8]
        t16 = te[C8 : min(ce, C)]
        n16 = len(t16)
        xt_full = np.zeros((D, C16), np.float16)
        xt_full[:, :n16] = xf[t16].T
        # chunk-packed swizzle: [p, (chunk kt c)]
        xtp = np.concatenate(
            [_swz(xt_full[:, c0 : c0 + S], KT1) for c0, S in CHUNKS16], axis=1
        )
        x8p = _swz(_q8(xf[t8].T, SX), KT1)
        w1f = W1[e].astype(np.float16)
        w1p = np.concatenate(
            [_swz(w1f[:, 0:1024], KT1), _swz(w1f[:, 1024:2048], KT1)], axis=1
        )
        w18f = _q8(W1[e], SW1)
        w18p = np.concatenate(
            [_swz(w18f[:, 0:1024], KT1), _swz(w18f[:, 1024:2048], KT1)], axis=1
        )
        w2p = _swz(W2[e].astype(np.float16), KT2)
        w28p = _swz(_q8(W2[e], SW2), KT2)
        bcp = np.zeros((P, FT + CT), np.float32)
        bcp[:, :FT] = b1[e].reshape(FT, P).T
        cwe = np.zeros((C,), np.float32)
        cwe[:C8] = cwk[e][:C8] * CW8_SCALE
        cwe[C8 : C8 + n16] = cwk[e][C8 : C8 + n16]
        # device token order is [fp16 block | fp8 block] in the y tensor
        cwdev = np.concatenate(
            [cwe[C8 : C8 + n16], np.zeros(C16 - n16, np.float32), cwe[:C8]]
        )
        bcp[:, FT:] = cwdev.reshape(CT, P).T
        in_maps.append(
            {
                "xt": xtp,
                "x8": x8p,
                "w1": w1p,
                "w2": w2p,
                "w18": w18p,
                "w28": w28p,
                "bc": bcp,
            }
        )

    nc = _build()
    trace = bool(os.environ.get("BASS_MOE_TRACE"))
    try:
        res = run_bass_kernel_spmd(
            nc,
            in_maps,
            core_ids=list(range(N_CORES)),
            trace=trace,
            trace_cores=list(range(N_CORES)) if trace else None,
        )
    except Exception:
        if not trace:
            raise
        trace = False
        res = run_bass_kernel_spmd(nc, in_maps, core_ids=list(range(N_CORES)))
    if trace and res.exec_time_ns is not None:
        print(f"HW exec time: {res.exec_time_ns} ns")
        print(f"mean exec time: {res.mean_exec_time_ns} ns")
        if res.instructions_and_trace is not None:
            print(f"trace: {res.instructions_and_trace[1]}")

    # ---- host combine: scatter-add expert outputs + cw-weighted b2 ----
    out = cw @ b2
    for e in range(E):
        te, ce = tok[e], len(tok[e])
        y = res.results[e]["y"].astype(np.float32)
        n16 = min(ce, C) - C8
        out[te[C8 : C8 + n16]] += y[:n16]
        out[te[:C8]] += y[C16:]
        th = te[C:]  # capacity-overflow tail: exact fp32 on host
        if len(th):
            yh = np.maximum(xf[th] @ W1[e] + b1[e], 0.0) @ W2[e]
            out[th] += cw[th, e][:, None] * yh
    return out.reshape(B, T, D)
